# revision 30
# baseline (speedup 1.0000x reference)
"""CSABlock Trainium2 kernel, plan E: act-roofline pipeline, local-first keys,
software-pipelined PE stream.

Core = 2n + h (sample n, image half h). Key px order per core is
[own 2048 | partner 2048] — attention is key-permutation-invariant, so each
core starts attention on its own half immediately while the partner half is
exchanged.

  - own h-half of feature[n] streamed as bf16, maxpool over D on DVE (2x),
  - ONE mask-zeroed 2-core ReduceScatter(add) exchanges the full pooled half
    (per-collective overhead is ~15us, so fewer is better); a warm-up
    collective at t=0 absorbs the NEFF's first-collective mesh setup,
  - theta (f32r, BN folded host-side) for own queries — second query half
    deferred so the PE never stalls on the late center half,
  - attention in two 1024-query passes, segment order p0-local, p1-local,
    p0-remote, p1-remote; within a segment the scores matmuls run 2 chunks
    ahead of the weighted matmuls so the PE stream never waits on the exp
    and stays at full p-state,
  - z accumulated as bf16 pair/quad/running adds on DVE through the pass,
  - tails at the end: z column-sum + 1/z broadcast on PE,
    reciprocal_approx_fast, normalize, out conv, residual, store.

PSUM: 2 pass accumulators (2 banks each) + a shared 2-deep [128,1024] ring
(theta/conv/scores/tails) = 8 banks exactly.
"""

import numpy as np
import ml_dtypes

import concourse.bass as bass
import concourse.mybir as mybir
import concourse.tile as tile
from concourse import bacc

F32 = mybir.dt.float32
F32R = mybir.dt.float32r
BF16 = mybir.dt.bfloat16

C = 256
CC = 2            # channel blocks of 128
IC = 128
D = 9
HW = 4096
Q = 2048          # local query/key pixels per core
NM = 4            # streamed pixel chunks of the local half
MPB = Q // NM     # 512 px per chunk
NCH = HW // 128   # 32 key chunks of 128 px over the full image
QP = 1024         # queries per attention pass
NPASS = 2
EXP_BIAS = -30.0
EPS = 1e-5
GROUPS = [[0, 1], [2, 3], [4, 5], [6, 7]]

AF = mybir.ActivationFunctionType
ALU = mybir.AluOpType


def build(nc):
    featd = nc.dram_tensor("feat", [CC, 128, NM, D, MPB], BF16, kind="ExternalInput")
    centerd = nc.dram_tensor("center", [CC, 2, 128, QP], F32R, kind="ExternalInput")
    wthd = nc.dram_tensor("wth", [CC, 128, 128], F32R, kind="ExternalInput")
    wphd = nc.dram_tensor("wph", [CC, 128, 128], BF16, kind="ExternalInput")
    wgd = nc.dram_tensor("wg", [CC, 128, 128], BF16, kind="ExternalInput")
    wwd = nc.dram_tensor("ww", [CC, 128, 128], F32R, kind="ExternalInput")
    bnbd = nc.dram_tensor("bnb", [128, 4], F32, kind="ExternalInput")
    outd = nc.dram_tensor("out", [CC, 128, Q], F32, kind="ExternalOutput")
    # exchange staging: slot s carries this core's pooled x masked for group
    # rank s (own slot zeroed); one RS(add) delivers the partner's full half.
    pbd = [nc.dram_tensor(f"pb{h}", [2, 128, CC, Q // 2], BF16) for h in range(2)]
    rsd = [nc.dram_tensor(f"rs{h}", [128, CC, Q // 2], BF16) for h in range(2)]
    # warm-up collective: absorbs the NEFF's first-collective setup latency.
    wui = nc.dram_tensor("wui", [128, 16], BF16)
    wuo = nc.dram_tensor("wuo", [2, 128, 16], BF16)

    with tile.TileContext(nc) as tc:
        with (
            tc.tile_pool(name="persist", bufs=1) as pp,
            tc.tile_pool(name="fstream", bufs=2) as fp,
            tc.tile_pool(name="mp", bufs=5) as mp,
            tc.tile_pool(name="et", bufs=8) as ep,
            tc.tile_pool(name="zt", bufs=4) as zp,
            tc.tile_pool(name="pk", bufs=2) as pk,
            tc.tile_pool(name="ot", bufs=2) as op,
            tc.tile_pool(name="psacc", bufs=2, space="PSUM") as pacc,
            tc.tile_pool(name="pssc", bufs=2, space="PSUM") as pss,
        ):
            nc.gpsimd.collective_compute(
                "AllGather", ALU.bypass, replica_groups=GROUPS,
                ins=[wui.ap().opt()], outs=[wuo.ap().opt()],
            )

            # ---- small loads; center arrives per query-half so pass-0 work
            # is never gated on the second half ----
            center_sb = pp.tile([128, CC, Q], F32R)
            wth = pp.tile([128, CC, 128], F32R)
            wph = pp.tile([128, CC, 128], BF16)
            wg = pp.tile([128, CC, 128], BF16)
            ww = pp.tile([128, CC, 128], F32R)
            bnb = pp.tile([128, 4], F32)
            for cc in range(CC):
                nc.sync.dma_start(
                    out=center_sb[:, cc, 0:QP], in_=centerd[cc, 0]
                )
            nc.sync.dma_start(out=wth[:, 0, :], in_=wthd[0])
            nc.sync.dma_start(out=wth[:, 1, :], in_=wthd[1])
            nc.sync.dma_start(out=wph[:, 0, :], in_=wphd[0])
            nc.sync.dma_start(out=wph[:, 1, :], in_=wphd[1])
            nc.sync.dma_start(out=wg[:, 0, :], in_=wgd[0])
            nc.sync.dma_start(out=wg[:, 1, :], in_=wgd[1])
            nc.sync.dma_start(out=ww[:, 0, :], in_=wwd[0])
            nc.sync.dma_start(out=ww[:, 1, :], in_=wwd[1])
            nc.sync.dma_start(out=bnb[:], in_=bnbd[:])

            expb = pp.tile([128, 1], F32)
            nc.gpsimd.memset(expb, EXP_BIAS)
            ones1b = pp.tile([1, 128], BF16)
            nc.gpsimd.memset(ones1b, 1.0)
            ones128b = pp.tile([128, 1], BF16)
            nc.gpsimd.memset(ones128b, 1.0)

            # ---- persistent state ----
            theta = pp.tile([128, Q], BF16)
            xall = pp.tile([128, CC, HW], BF16)   # [ch, cc, px]; own | partner
            phi = pp.tile([128, HW], BF16)
            gT = pp.tile([128, NCH, 128], BF16)   # [px-in-chunk, chunk, ch]
            zrowb = pp.tile([1, NPASS, QP], BF16)
            wsb = pp.tile([128, NPASS, QP], F32R)

            def theta_half(qh):
                ps_t = pss.tile([128, QP], F32, tag="sc", name="ps_t")
                for cc in range(CC):
                    for qc in range(2):
                        nc.tensor.matmul(
                            ps_t[:, qc * 512 : (qc + 1) * 512],
                            lhsT=wth[:, cc, :],
                            rhs=center_sb[:, cc, qh * QP + qc * 512 : qh * QP + (qc + 1) * 512],
                            start=(cc == 0),
                            stop=(cc == 1),
                        )
                nc.scalar.activation(
                    theta[:, qh * QP : (qh + 1) * QP], ps_t,
                    AF.Relu, bias=bnb[:, 0:1],
                )

            theta_half(0)

            # ---- attention machinery ----
            st = {
                p: {"acc": None, "ets": [], "prs": [], "quads": [], "zacc": None}
                for p in range(NPASS)
            }

            def drain_z(p):
                """fold pending pairs into quads and the running zacc."""
                s = st[p]
                while len(s["prs"]) >= 2:
                    qd = zp.tile([128, QP], BF16, tag="quad", bufs=4, name="qd")
                    nc.vector.tensor_add(qd, s["prs"][0], s["prs"][1])
                    s["prs"] = s["prs"][2:]
                    s["quads"].append(qd)
                    if len(s["quads"]) == 2:
                        q0, q1 = s["quads"]
                        s["quads"] = []
                        if s["zacc"] is None:
                            za = pp.tile([128, QP], BF16, name=f"zacc{p}")
                            s["zacc"] = za
                            nc.vector.tensor_add(za, q0, q1)
                        else:
                            tq = zp.tile([128, QP], BF16, tag="quad", bufs=4, name="tq")
                            nc.vector.tensor_add(tq, q0, q1)
                            nc.vector.tensor_add(s["zacc"], s["zacc"], tq)
            wq = []  # pending weighted matmuls: (p, c, et, first, last)

            def conv_chunk(off, chi):
                """phi/gT for 512 px starting at global px `off` (chunk chi*4)."""
                ps_p = pss.tile([128, QP], F32, tag="sc", name="ps_p")
                for cc in range(CC):
                    nc.tensor.matmul(
                        ps_p[:, 0:MPB],
                        lhsT=wph[:, cc, :],
                        rhs=xall[:, cc, off : off + MPB],
                        start=(cc == 0),
                        stop=(cc == 1),
                    )
                nc.scalar.activation(
                    phi[:, off : off + MPB], ps_p[:, 0:MPB],
                    AF.Relu, bias=bnb[:, 1:2],
                )
                ps_g = pss.tile([128, QP], F32, tag="sc", name="ps_g")
                for j in range(4):
                    for cc in range(CC):
                        nc.tensor.matmul(
                            ps_g[:, j * 128 : (j + 1) * 128],
                            lhsT=xall[:, cc, off + j * 128 : off + (j + 1) * 128],
                            rhs=wg[:, cc, :],
                            start=(cc == 0),
                            stop=(cc == 1),
                        )
                nc.vector.tensor_copy(gT[:, chi * 4 : chi * 4 + 4, :], ps_g[:, 0:MPB])

            def emit_weighted(p, c, et, first, last):
                for qc in range(2):
                    nc.tensor.matmul(
                        st[p]["acc"][:, qc * 512 : (qc + 1) * 512],
                        lhsT=gT[:, c, :],
                        rhs=et[:, qc * 512 : (qc + 1) * 512],
                        start=first,
                        stop=last,
                    )

            def att_chunk(p, c, first, last):
                """scores -> exp (weighted deferred 2 chunks) + z adds."""
                s = st[p]
                s_ps = pss.tile([128, QP], F32, tag="sc", name="s_ps")
                for qc in range(2):
                    nc.tensor.matmul(
                        s_ps[:, qc * 512 : (qc + 1) * 512],
                        lhsT=phi[:, c * 128 : (c + 1) * 128],
                        rhs=theta[:, p * QP + qc * 512 : p * QP + (qc + 1) * 512],
                        start=True,
                        stop=True,
                    )
                et = ep.tile([128, QP], BF16, tag="et")
                nc.scalar.activation(et, s_ps, AF.Exp, bias=expb[:])
                wq.append((p, c, et, first, last))
                if len(wq) > 2:
                    emit_weighted(*wq.pop(0))
                s["ets"].append(et)
                if len(s["ets"]) == 2:
                    pr = zp.tile([128, QP], BF16, tag="pair", bufs=4)
                    nc.vector.tensor_add(pr, s["ets"][0], s["ets"][1])
                    s["ets"] = []
                    s["prs"].append(pr)
                drain_z(p)

            def flush_wq():
                while wq:
                    emit_weighted(*wq.pop(0))

            def pass_tail(p):
                """z colsum + 1/z broadcast + normalize + out conv + store."""
                s = st[p]
                zrow = pss.tile([128, QP], F32, tag="sc", name="zrow")
                for qc in range(2):
                    nc.tensor.matmul(
                        zrow[0:1, qc * 512 : (qc + 1) * 512],
                        lhsT=ones128b[:, 0:1],
                        rhs=s["zacc"][:, qc * 512 : (qc + 1) * 512],
                        start=True,
                        stop=True,
                    )
                nc.vector.tensor_copy(zrowb[:, p, :], zrow[0:1, :])
                bps = pss.tile([128, QP], F32, tag="sc", name="bps")
                for qc in range(2):
                    nc.tensor.matmul(
                        bps[:, qc * 512 : (qc + 1) * 512],
                        lhsT=ones1b[0:1, :],
                        rhs=zrowb[:, p, qc * 512 : (qc + 1) * 512],
                        start=True,
                        stop=True,
                    )
                invbc = op.tile([128, QP], F32, tag="bps", bufs=2)
                nc.vector.reciprocal_approx_fast(out=invbc, in_=bps)
                nc.vector.tensor_mul(wsb[:, p, :], s["acc"], invbc)
                for oc in range(CC):
                    pso = pss.tile([128, QP], F32, tag="sc", name="pso")
                    for qc in range(2):
                        nc.tensor.matmul(
                            pso[:, qc * 512 : (qc + 1) * 512],
                            lhsT=ww[:, oc, :],
                            rhs=wsb[:, p, qc * 512 : (qc + 1) * 512],
                            start=True,
                            stop=True,
                        )
                    osb = op.tile([128, QP], F32, tag="ot")
                    nc.vector.tensor_add(
                        osb, pso, center_sb[:, oc, p * QP : (p + 1) * QP]
                    )
                    nc.sync.dma_start(
                        out=outd[oc][:, p * QP : (p + 1) * QP], in_=osb
                    )

            # ---- m-loop: stream local feature, maxpool, conv, pass-0 local
            # chunks; masked staging trails the compute ----
            st[0]["acc"] = pacc.tile([128, QP], F32, tag="acc", name="acc0")
            for m in range(NM):
                ft = fp.tile([128, CC, D, MPB], BF16, tag="ft")
                for cc in range(CC):
                    nc.sync.dma_start(out=ft[:, cc], in_=featd[cc, :, m])
                t_a = mp.tile([128, CC, MPB], BF16, tag="mp")
                t_b = mp.tile([128, CC, MPB], BF16, tag="mp")
                t_c = mp.tile([128, CC, MPB], BF16, tag="mp")
                t_d = mp.tile([128, CC, MPB], BF16, tag="mp")
                nc.vector.tensor_max(t_a, ft[:, :, 0, :], ft[:, :, 1, :])
                nc.vector.tensor_max(t_b, ft[:, :, 2, :], ft[:, :, 3, :])
                nc.vector.tensor_max(t_c, ft[:, :, 4, :], ft[:, :, 5, :])
                nc.vector.tensor_max(t_d, ft[:, :, 6, :], ft[:, :, 7, :])
                nc.vector.tensor_max(t_a, t_a, t_b)
                nc.vector.tensor_max(t_c, t_c, t_d)
                nc.vector.tensor_max(t_a, t_a, t_c)
                nc.vector.tensor_max(
                    xall[:, :, m * MPB : (m + 1) * MPB], t_a, ft[:, :, 8, :]
                )
                conv_chunk(m * MPB, m)
                for c in range(4 * m, 4 * m + 4):
                    att_chunk(0, c, first=(c == 0), last=False)
                # masked staging for the exchange (on the scalar engine,
                # which has slack here; Copy+scale needs no act table)
                pkm = pk.tile([128, 2, 2 * MPB], BF16, tag="pk")
                for sl in range(2):
                    for cc in range(CC):
                        nc.scalar.mul(
                            pkm[:, sl, cc * MPB : (cc + 1) * MPB],
                            xall[:, cc, m * MPB : (m + 1) * MPB],
                            bnb[:, 2 + sl : 3 + sl],
                        )
                    nc.gpsimd.dma_start(
                        out=pbd[m // 2][sl, :, :, (m % 2) * MPB : (m % 2 + 1) * MPB],
                        in_=pkm[:, sl, :].rearrange("p (cc q) -> p cc q", cc=CC),
                    )
                if m % 2 == 1:
                    h = m // 2
                    nc.gpsimd.collective_compute(
                        "ReduceScatter", ALU.add, replica_groups=GROUPS,
                        ins=[pbd[h].ap().opt()], outs=[rsd[h].ap().opt()],
                    )
                    for cc in range(CC):
                        nc.gpsimd.dma_start(
                            out=xall[:, cc, Q + h * QP : Q + (h + 1) * QP],
                            in_=rsd[h][:, cc, :],
                        )

            # ---- pass-1 local chunks (all local data resident) ----
            flush_wq()
            for cc in range(CC):
                nc.sync.dma_start(
                    out=center_sb[:, cc, QP : 2 * QP], in_=centerd[cc, 1]
                )
            theta_half(1)
            st[1]["acc"] = pacc.tile([128, QP], F32, tag="acc", name="acc1")
            for c in range(16):
                att_chunk(1, c, first=(c == 0), last=False)

            # ---- remote chunks (gated on the exchange) ----
            flush_wq()
            for rm in range(NM):
                conv_chunk(Q + rm * MPB, NM + rm)
                for c in range(16 + 4 * rm, 16 + 4 * rm + 4):
                    att_chunk(0, c, first=False, last=(c == NCH - 1))
            flush_wq()
            for c in range(16, NCH):
                att_chunk(1, c, first=False, last=(c == NCH - 1))
            flush_wq()
            pass_tail(0)
            pass_tail(1)


def shard_inputs(inputs):
    f32 = np.float32
    bf16 = ml_dtypes.bfloat16
    feature = np.asarray(inputs["feature"], dtype=f32)
    w_theta = np.asarray(inputs["w_theta"], dtype=f32)
    w_phi = np.asarray(inputs["w_phi"], dtype=f32)
    w_g = np.asarray(inputs["w_g"], dtype=f32)
    w_w = np.asarray(inputs["w_w"], dtype=f32)

    # fold BN (inference) into the conv weights: y = W'x + b'
    sc_th = np.asarray(inputs["bn_theta_gamma"], f32) / np.sqrt(
        np.asarray(inputs["bn_theta_var"], f32) + EPS
    )
    b_th = np.asarray(inputs["bn_theta_beta"], f32) - np.asarray(
        inputs["bn_theta_mean"], f32
    ) * sc_th
    sc_ph = np.asarray(inputs["bn_phi_gamma"], f32) / np.sqrt(
        np.asarray(inputs["bn_phi_var"], f32) + EPS
    )
    b_ph = np.asarray(inputs["bn_phi_beta"], f32) - np.asarray(
        inputs["bn_phi_mean"], f32
    ) * sc_ph

    wth = np.ascontiguousarray((w_theta * sc_th[:, None]).T.reshape(2, 128, 128))
    wph = np.ascontiguousarray(
        (w_phi * sc_ph[:, None]).T.reshape(2, 128, 128)
    ).astype(bf16)
    wgT = np.ascontiguousarray(w_g.T.reshape(2, 128, 128)).astype(bf16)
    wwT = np.ascontiguousarray(w_w.T.reshape(128, 2, 128).transpose(1, 0, 2))

    in_maps = []
    for core in range(8):
        n, h = core // 2, core % 2
        fh = feature[n].reshape(2, 128, D, HW)[:, :, :, h * Q : (h + 1) * Q]
        feat = np.ascontiguousarray(
            fh.reshape(2, 128, D, NM, MPB).transpose(0, 1, 3, 2, 4).astype(bf16)
        )
        center = np.ascontiguousarray(
            feature[n][:, D // 2 + 1].reshape(256, HW)[:, h * Q : (h + 1) * Q]
            .reshape(2, 128, 2, QP).transpose(0, 2, 1, 3)
        )
        bnb = np.zeros((128, 4), dtype=f32)
        bnb[:, 0] = b_th
        bnb[:, 1] = b_ph
        bnb[:, 2 + (1 - h)] = 1.0
        in_maps.append(
            dict(feat=feat, center=center, wth=wth, wph=wph, wg=wgT,
                 ww=wwT, bnb=bnb)
        )
    return in_maps


def unshard_output(results, N=4):
    out = np.empty((N, 256, 64, 64), dtype=np.float32)
    flat = out.reshape(N, 256, HW)
    for core in range(8):
        n, qh = core // 2, core % 2
        flat[n][:, qh * Q : (qh + 1) * Q] = results[core]["out"].reshape(256, Q)
    return out


def make_nc():
    nc = bacc.Bacc("TRN2", target_bir_lowering=False, debug=False, num_devices=8)
    build(nc)
    nc.compile()
    return nc


# ---------------------------------------------------------------------------
# Public entrypoint: full (unsharded) inputs -> full output, running the Bass
# kernel SPMD across the 8 NeuronCores.
# ---------------------------------------------------------------------------
from concourse.bass_utils import run_bass_kernel_spmd

_NC_CACHE = []


def _get_nc():
    if not _NC_CACHE:
        _NC_CACHE.append(make_nc())
    return _NC_CACHE[0]


def kernel(**inputs):
    nc = _get_nc()
    in_maps = shard_inputs(inputs)
    res = run_bass_kernel_spmd(nc, in_maps, list(range(8)))
    return unshard_output(res.results)
